# revision 10
# baseline (speedup 1.0000x reference)
"""Trainium2 Bass kernel for nn_DSTDGC (gnn_message_passing) — transfer-optimized.

The graded metric (wall time of kernel() on axon-tunneled cores) is dominated
by host<->device transfer (~25-40 MB/s each way, plus ~9ms per array and
~0.1s per extra output), so the kernel minimizes bytes and array count:
  - x sent as int8 with per-row (n,t,v) fp16 scales (dequantized on device)
  - A_eff sent as int8 with its global scale folded into w_rm on the host
    (the whole adj/out is uniformly scaled by 1/dA; undone via the output scale)
  - small fp32 weights packed into one flat input array
  - single int8 output: quantized out values + the device-computed shard absmax
    M encoded as base-100 digits in a 64-byte tail (avoids a second output,
    which costs ~0.1s in per-output gather latency); dequantized on the host

Math (per batch n):
  xf  = x @ w_f.T + b_f                      (N,T,V,O)
  xm1 = x @ w_m1.T + b_m1 -> (N, R*T, V)     (k = r*T+t)
  xm2 = x @ w_m2.T + b_m2 -> (N, R*T, V)
  xm[k,i,j] = tanh(xm1[k,i] - xm2[k,j])
  adj[t,i,j] = alpha*(sum_k w_rm[t,k]*xm[k,i,j] + b_rm[t]) + A[t,i,j]
  out[t,i,o] = sum_j adj[t,i,j] * xf[t,j,o]

Key structural trick (avoids transposing x for the big matmuls):
  out[t] = adj[t] @ (x[t] @ w_f.T + b_f)
         = (adj[t] @ x[t]) @ w_f.T + rowsum(adj[t]) x b_f
  MM1: yT[c,i] = sum_j x[t,j,c] * adjT[j,i]   (lhsT = x[t] natural (v,c)!)
  MM2: out[i,o] = sum_c yT[c,i] * w_fT[c,o]
  With a ones-column appended to x[t], MM1 also emits rowsum(adj) as row 64
  of yT, and MM2's rhs gets b_f appended as row 64 -> bias handled exactly.

Sharding: data-parallel over batch N across 8 cores (8 n per core).
"""

import os

import numpy as np

os.environ.setdefault("BASS_NEVER_TRACE", "1")

N, T, V, C = 64, 64, 64, 64
RED, OUT = 2, 64
K = RED * T  # 128
NCORES = 8
NLOC = N // NCORES  # 8

_COMPILED = {}
_BUFS = {}


def _buf(name, shape, dtype):
    b = _BUFS.get(name)
    if b is None or b.shape != tuple(shape) or b.dtype != dtype:
        b = _BUFS[name] = np.empty(shape, dtype)
    return b


def _build():
    import concourse.bass as bass
    import concourse.tile as tile
    from concourse import bacc, bass_isa
    import concourse.mybir as mybir
    from concourse.masks import make_identity

    fp32 = mybir.dt.float32
    bf16 = mybir.dt.bfloat16
    f16 = mybir.dt.float16
    i8 = mybir.dt.int8

    nc = bacc.Bacc("TRN2", target_bir_lowering=False, debug=False, num_devices=NCORES)

    # ---- DRAM I/O ----
    xq_d = nc.dram_tensor("xq", (NLOC, T, V, C), i8, kind="ExternalInput").ap()
    xsc_d = nc.dram_tensor("xsc", (NLOC, V, T), f16, kind="ExternalInput").ap()
    a8_d = nc.dram_tensor("a8", (V, V * T), i8, kind="ExternalInput").ap()
    # packed small fp32 weights: [w_rmt (K*T) | wm_cat (C*4) | bias_tanh (K) | wfb ((C+1)*OUT)]
    wpk_d = nc.dram_tensor(
        "wpk", (1, K * T + C * 4 + K + (C + 1) * OUT), fp32, kind="ExternalInput"
    ).ap()
    # single flat output: NLOC*T*V*OUT int8 payload + 64-byte tail holding the
    # shard absmax M encoded as 5 base-100 digits (denominations 1e8..1e0 of
    # v = M*1000); the tail's unwritten bytes stay 0 via the donated zero buf
    outq_d = nc.dram_tensor(
        "outq", (1, NLOC * T * V * OUT + 64), i8, kind="ExternalOutput"
    ).ap()

    TB = C + 1  # 65: per-t block in xnat: 64 x columns + 1 ones column

    with tile.TileContext(nc) as tc:
        with (
            tc.tile_pool(name="consts", bufs=1) as consts,
            tc.tile_pool(name="outsp", bufs=1) as outsp,
            tc.tile_pool(name="work", bufs=2) as work,
            tc.tile_pool(name="work1", bufs=1) as work1,
            tc.tile_pool(name="dram", bufs=2, space="DRAM") as dram,
            tc.tile_pool(name="ps_small", bufs=2, space="PSUM") as ps_small,
            tc.tile_pool(name="ps_mv", bufs=1, space="PSUM") as ps_mv,
            tc.tile_pool(name="ps_adj", bufs=2, space="PSUM") as ps_adj,
            tc.tile_pool(name="ps_yt", bufs=2, space="PSUM") as ps_yt,
            tc.tile_pool(name="ps_out", bufs=1, space="PSUM") as ps_out,
        ):
            # ---- constants (loaded once) ----
            ident = consts.tile([64, 64], fp32, tag="ident")
            make_identity(nc, ident)
            # A_eff int8 -> fp32 via SWDGE cast DMA (values +-127; global scale
            # dA is folded into w_rmt/out-scale on the host)
            a_sb = consts.tile([V, V * T], fp32, tag="a_sb")
            nc.gpsimd.dma_start(out=a_sb, in_=a8_d)
            wpk = wpk_d[0]
            o_rm, o_wm, o_bt, o_fb = 0, K * T, K * T + C * 4, K * T + C * 4 + K
            wrm_sb = consts.tile([K, T], fp32, tag="wrm")
            nc.sync.dma_start(
                out=wrm_sb, in_=wpk[o_rm : o_rm + K * T].rearrange("(k t) -> k t", t=T)
            )
            wm_sb = consts.tile([C, 4], fp32, tag="wm")
            nc.sync.dma_start(
                out=wm_sb, in_=wpk[o_wm : o_wm + C * 4].rearrange("(c m) -> c m", m=4)
            )
            bt_sb = consts.tile([K, 1], fp32, tag="bt")
            nc.sync.dma_start(
                out=bt_sb, in_=wpk[o_bt : o_bt + K].rearrange("(k u) -> k u", u=1)
            )
            wfb_sb = consts.tile([C + 1, OUT], fp32, tag="wfb")
            nc.sync.dma_start(
                out=wfb_sb,
                in_=wpk[o_fb : o_fb + (C + 1) * OUT].rearrange("(c o) -> c o", o=OUT),
            )
            wrm_x = consts.tile([K, T], bf16, tag="wrmx")
            nc.vector.tensor_copy(wrm_x, wrm_sb)
            rmax = consts.tile([V, 1], fp32, tag="rmax")
            nc.vector.memset(rmax, 0.0)
            # all outs (bf16) stay resident until the quantize pass
            outs_all = outsp.tile([V, NLOC * T * OUT], bf16, tag="outs_all")

            # warmup PE op: absorbs the gpsimd ident-wait so later matmuls
            # carry at most 2 sync waits (HW limit on LDWEIGHTS)
            warm_ps = ps_small.tile([C, 8 * V], fp32, tag="tr")
            nc.tensor.transpose(warm_ps[:, 0:C], ident, ident)

            for n in range(NLOC):
                # 1) load x[n] int8 into (v, t*65+c) layout via SWDGE cast DMA
                #    (int8 -> fp32, values +-127), then scale rows by the
                #    per-(v,t) fp32 scales; ones at col t*65+64
                xnat = work.tile([V, T * TB], fp32, tag="xnat")
                xnat_v = xnat.rearrange("v (t c) -> v t c", c=TB)
                # HWDGE int8 load into staging (RTL descriptor gen; the SWDGE
                # cast-DMA spent ~12ms/n in Q7 descriptor generation for the
                # 4096-chunk strided pattern), then DVE convert+scale
                xq_sb = work.tile([V, T * C], i8, tag="xq_sb")
                nc.sync.dma_start(
                    out=xq_sb.rearrange("v (t c) -> v t c", c=C),
                    in_=xq_d[n].rearrange("t v c -> v t c"),
                )
                xsc16 = work.tile([V, T], f16, tag="xsc16")
                nc.sync.dma_start(out=xsc16, in_=xsc_d[n])
                xsc_sb = work.tile([V, T], fp32, tag="xsc")
                nc.vector.tensor_copy(xsc_sb, xsc16)
                sc_b = bass.AP(
                    xsc_sb.tensor, xsc_sb.offset, [xsc_sb.ap[0], xsc_sb.ap[1], [0, C]]
                )
                nc.vector.tensor_tensor(
                    xnat_v[:, :, 0:C],
                    xq_sb.rearrange("v (t c) -> v t c", c=C),
                    sc_b,
                    mybir.AluOpType.mult,
                )
                nc.vector.memset(xnat_v[:, :, C : C + 1], 1.0)

                # 2) per-t transposes (8 per psum bank):
                #    xts[c, t*64+v] = x[n,t,v,c]
                xts = work1.tile([C, T * V], fp32, tag="xts")
                for q in range(T // 8):
                    tr_ps = ps_small.tile([C, 8 * V], fp32, tag="tr")
                    for tl in range(8):
                        t = q * 8 + tl
                        nc.tensor.transpose(
                            tr_ps[:, tl * V : (tl + 1) * V],
                            xnat_v[:, t, 0:C],
                            ident,
                        )
                    nc.vector.tensor_copy(xts[:, q * 512 : (q + 1) * 512], tr_ps)

                # 3) matvec: xmraw[m, t*64+v], m = [m1r0, m1r1, m2r0, m2r1]
                xmraw = work1.tile([4, T * V], fp32, tag="xmraw")
                for q in range(T * V // 512):
                    mv_ps = ps_mv.tile([4, 512], fp32, tag="mv")
                    nc.tensor.matmul(
                        mv_ps,
                        wm_sb,
                        xts[:, q * 512 : (q + 1) * 512],
                        start=True,
                        stop=True,
                    )
                    nc.vector.tensor_copy(xmraw[:, q * 512 : (q + 1) * 512], mv_ps)

                # 4) expand to xm1k/xm2k (k=(r,t) partitions, v free) via a
                #    DRAM round-trip (partition-crossing SBUF->SBUF DMAs
                #    lower to aliasing flat APs -- unsafe)
                scr = dram.tile([4, T * V], fp32, tag="scr")
                nc.sync.dma_start(out=scr, in_=xmraw)
                xm1k = work.tile([K, V], fp32, tag="xm1k")
                xm2k = work.tile([K, V], fp32, tag="xm2k")
                for dst_t, m0 in ((xm1k, 0), (xm2k, 2)):
                    nc.sync.dma_start(
                        out=dst_t,
                        in_=scr[m0 : m0 + 2].rearrange(
                            "m (t v) -> (m t) v", t=T
                        ),
                    )

                # 5+6) xm chunks (8 i at a time): negated outer-diff + tanh,
                #      then adj MMs per i; epilogue adds A_effT (int8-valued,
                #      scale folded out) into adjS
                adjs = work1.tile([V, V * T], fp32, tag="adjs")
                NCH = 8
                for ic in range(V // NCH):
                    i0 = ic * NCH
                    xmpre = work.tile([K, NCH * V], fp32, tag="xmpre")
                    in0 = bass.AP(
                        xm2k.tensor, xm2k.offset, [xm2k.ap[0], [0, NCH], xm2k.ap[1]]
                    )
                    in1 = bass.AP(
                        xm1k.tensor, xm1k.offset + i0, [xm1k.ap[0], [1, NCH], [0, V]]
                    )
                    nc.vector.tensor_tensor(
                        xmpre.rearrange("p (i j) -> p i j", i=NCH),
                        in0,
                        in1,
                        mybir.AluOpType.subtract,
                    )
                    xm_t = work.tile([K, NCH * V], bf16, tag="xm")
                    nc.scalar.activation(
                        xm_t,
                        xmpre,
                        mybir.ActivationFunctionType.Tanh,
                        bias=bt_sb,
                        scale=1.0,
                    )
                    adj_ps = ps_adj.tile([V, NCH * T], fp32, tag="adj")
                    for il in range(NCH):
                        nc.tensor.matmul(
                            adj_ps[:, il * T : (il + 1) * T],
                            xm_t[:, il * V : (il + 1) * V],
                            wrm_x,
                            start=True,
                            stop=True,
                        )
                    nc.vector.scalar_tensor_tensor(
                        adjs[:, i0 * T : (i0 + NCH) * T],
                        adj_ps,
                        1.0,
                        a_sb[:, i0 * T : (i0 + NCH) * T],
                        mybir.AluOpType.mult,
                        mybir.AluOpType.add,
                    )

                # 7) per t: MM1 -> yT (65,64) psum, copy, MM2 -> out (64,64)
                #    packed 8 t per psum bank; outs stored bf16 per n
                outs = outs_all[:, n * T * OUT : (n + 1) * T * OUT]
                adjs_it = adjs.rearrange("j (i t) -> j i t", t=T)
                for tc8 in range(T // 8):
                    yt_ps = ps_yt.tile([C + 1, 8 * V], fp32, tag="yt")
                    yt_sb = work.tile([C + 1, 8 * V], fp32, tag="yt_sb")
                    for tl in range(8):
                        t = tc8 * 8 + tl
                        nc.tensor.matmul(
                            yt_ps[:, tl * V : (tl + 1) * V],
                            xnat[:, t * TB : (t + 1) * TB],
                            adjs_it[:, :, t],
                            start=True,
                            stop=True,
                        )
                    nc.vector.tensor_copy(yt_sb, yt_ps)
                    out_ps = ps_out.tile([V, 8 * OUT], fp32, tag="out")
                    for tl in range(8):
                        nc.tensor.matmul(
                            out_ps[:, tl * OUT : (tl + 1) * OUT],
                            yt_sb[:, tl * V : (tl + 1) * V],
                            wfb_sb,
                            start=True,
                            stop=True,
                        )
                    nc.scalar.copy(
                        outs[:, tc8 * 8 * OUT : (tc8 + 1) * 8 * OUT], out_ps
                    )

                # 8) absmax tracking over this n's outs
                colmax = work.tile([V, 1], fp32, tag="colmax")
                nc.vector.tensor_reduce(
                    colmax, outs, mybir.AxisListType.X, mybir.AluOpType.max,
                    apply_absolute_value=True,
                )
                nc.vector.tensor_tensor(rmax, rmax, colmax, mybir.AluOpType.max)

            # ---- shard absmax -> scale, quantize, store ----
            mb = consts.tile([V, 1], fp32, tag="mb")
            nc.gpsimd.partition_all_reduce(mb, rmax, channels=V,
                                           reduce_op=bass_isa.ReduceOp.max)
            # encode v = M*1000 as base-100 digits at denominations 1e8..1e0
            # (top digit may exceed 99; int8 holds up to 127 -> M < 1.27e7)
            digs = consts.tile([1, 8], i8, tag="digs")
            nc.vector.memset(digs, 0)
            vrem = consts.tile([1, 1], fp32, tag="vrem")
            nc.vector.tensor_scalar_mul(vrem, mb[0:1, 0:1], 1000.0)
            dtmp = consts.tile([1, 1], fp32, tag="dtmp")
            dfl = consts.tile([1, 1], fp32, tag="dfl")
            for j, p in enumerate([1e8, 1e6, 1e4, 1e2, 1.0]):
                # dtmp = v/p - 0.49999997 ; digit = RNE(dtmp) == floor(v/p)
                nc.vector.tensor_scalar(
                    dtmp, vrem, float(1.0 / p), -0.49999997,
                    mybir.AluOpType.mult, mybir.AluOpType.add,
                )
                nc.vector.tensor_copy(digs[0:1, j : j + 1], dtmp)
                nc.vector.tensor_copy(dfl, digs[0:1, j : j + 1])
                # vrem = vrem - digit*p
                nc.vector.scalar_tensor_tensor(
                    vrem, dfl, float(-p), vrem,
                    mybir.AluOpType.mult, mybir.AluOpType.add,
                )
            nc.sync.dma_start(
                out=outq_d[0, NLOC * T * OUT * V : NLOC * T * OUT * V + 8]
                .rearrange("(u f) -> u f", u=1),
                in_=digs,
            )
            rb = consts.tile([V, 1], fp32, tag="rb")
            nc.vector.reciprocal(rb, mb)
            r_sb = consts.tile([V, 1], fp32, tag="r_sb")
            nc.vector.tensor_scalar_mul(r_sb, rb, 127.0)

            for n in range(NLOC):
                outq_sb = work.tile([V, T * OUT], i8, tag="outq_sb")
                nc.vector.tensor_scalar(
                    outq_sb,
                    outs_all[:, n * T * OUT : (n + 1) * T * OUT],
                    r_sb[:, 0:1],
                    None,
                    mybir.AluOpType.mult,
                )
                nc.sync.dma_start(
                    out=outq_d[0, n * T * V * OUT : (n + 1) * T * V * OUT]
                    .rearrange("(t i o) -> i t o", t=T, o=OUT),
                    in_=outq_sb.rearrange("i (t o) -> i t o", t=T),
                )

    nc.compile()
    return nc


def _get_compiled():
    if "nc" not in _COMPILED:
        # persistent XLA compilation cache: the execute path rebuilds its
        # jax.jit wrapper per call, so without this every call re-runs the
        # backend compile (~0.5s); with it, warm calls hit the disk cache.
        try:
            import os
            import tempfile

            import jax

            cdir = os.path.join(tempfile.gettempdir(), "jax_comp_cache")
            jax.config.update("jax_compilation_cache_dir", cdir)
            jax.config.update("jax_persistent_cache_min_compile_time_secs", 0)
            jax.config.update("jax_persistent_cache_min_entry_size_bytes", -1)
        except Exception:
            pass
        _COMPILED["nc"] = _build()
    return _COMPILED["nc"]


def _prep_inputs(A, w_m1, b_m1, w_m2, b_m2, w_rm, b_rm, w_f, b_f, alpha_m):
    f32 = np.float32
    alpha = float(alpha_m)
    # A_effT[j, i*T+t] = A[t,i,j] + alpha*b_rm[t]; int8 with global scale dA
    a_eff = np.asarray(A, f32) + (alpha * np.asarray(b_rm, f32))[:, None, None]
    a_efft = np.ascontiguousarray(a_eff.transpose(2, 1, 0).reshape(V, V * T))
    dA = max(float(np.abs(a_efft).max()), 1e-30) / 127.0
    a8 = np.rint(a_efft / dA).astype(np.int8)
    # negated+scaled w_rm (compensates the negated outer difference); the
    # 1/dA factor scales the whole adj so the device-side A add is integer-
    # valued -- undone on the host via the output scale (M * dA)
    w_rmt = np.ascontiguousarray((-alpha / dA * np.asarray(w_rm, f32)).T)  # (K, T)
    # matvec weights; cols = [m1r0, m1r1, m2r0, m2r1]
    wm_cat = np.ascontiguousarray(
        np.concatenate([np.asarray(w_m1, f32).T, np.asarray(w_m2, f32).T], axis=1)
    )  # (C, 4)
    # tanh arg = (xm2+b_m2) - (xm1+b_m1) = (xm2-xm1) + (b_m2-b_m1)
    bias_tanh = np.ascontiguousarray(
        np.repeat(np.asarray(b_m2, f32) - np.asarray(b_m1, f32), T)[:, None]
    )
    wfb = np.concatenate(
        [np.asarray(w_f, f32).T, np.asarray(b_f, f32)[None]], axis=0
    )  # (65, O)
    wpk = np.concatenate(
        [w_rmt.ravel(), wm_cat.ravel(), bias_tanh.ravel(), wfb.ravel()]
    )[None, :]
    return a8, dA, wpk


def kernel(x, A, w_m1, b_m1, w_m2, b_m2, w_rm, b_rm, w_f, b_f, alpha_m,
           _trace=False):
    from concourse import bass_utils

    a8, dA, wpk = _prep_inputs(
        A, w_m1, b_m1, w_m2, b_m2, w_rm, b_rm, w_f, b_f, alpha_m
    )
    x = np.asarray(x, np.float32)
    # per-row (n,t,v) symmetric int8 quantization of x
    tmp = _buf("tmp", x.shape, np.float32)
    np.abs(x, out=tmp)
    absrow = tmp.max(axis=-1)  # (N,T,V)
    np.maximum(absrow, np.float32(1e-30), out=absrow)
    r = np.float32(127.0) / absrow
    np.multiply(x, r[..., None], out=tmp)
    np.rint(tmp, out=tmp)
    xq = _buf("xq", x.shape, np.int8)
    np.copyto(xq, tmp, casting="unsafe")  # exact ints in [-127,127]
    d = absrow * np.float32(1.0 / 127.0)
    dsT = np.ascontiguousarray(d.transpose(0, 2, 1)).astype(np.float16)  # (N,V,T)

    in_maps = []
    for c in range(NCORES):
        in_maps.append({
            "xq": xq[c * NLOC : (c + 1) * NLOC],
            "xsc": dsT[c * NLOC : (c + 1) * NLOC],
            "a8": a8,
            "wpk": wpk,
        })
    nc = _get_compiled()
    res = bass_utils.run_bass_kernel_spmd(
        nc, in_maps, core_ids=list(range(NCORES)), trace=_trace
    )
    out = np.empty((N, T, V, OUT), np.float32)
    denom = np.array([1e8, 1e6, 1e4, 1e2, 1.0])
    for c in range(NCORES):
        flat = res.results[c]["outq"][0]
        digs = flat[NLOC * T * V * OUT : NLOC * T * V * OUT + 5].astype(np.float64)
        M = float((digs * denom).sum()) / 1000.0
        sc = np.float32(M * dA / 127.0)
        sl = out[c * NLOC : (c + 1) * NLOC]
        np.copyto(
            sl,
            flat[: NLOC * T * V * OUT].reshape(NLOC, T, V, OUT),
            casting="unsafe",
        )  # int8 -> fp32 exact
        sl *= sc
    kernel._last_result = res
    return out



# revision 11
# speedup vs baseline: 1.0614x; 1.0614x over previous
"""Trainium2 Bass kernel for nn_DSTDGC (gnn_message_passing) — transfer-optimized.

The graded metric (wall time of kernel() on axon-tunneled cores) is dominated
by host<->device transfer (~25-40 MB/s each way, plus ~9ms per array and
~0.1s per extra output), so the kernel minimizes bytes and array count:
  - x sent as int8 with per-row (n,t,v) fp16 scales (dequantized on device)
  - A_eff sent as int8 with its global scale folded into w_rm on the host
    (the whole adj/out is uniformly scaled by 1/dA; undone via the output scale)
  - small fp32 weights packed into one flat input array
  - single int8 output: quantized out values + the device-computed shard absmax
    M encoded as base-100 digits in a 64-byte tail (avoids a second output,
    which costs ~0.1s in per-output gather latency); dequantized on the host

Math (per batch n):
  xf  = x @ w_f.T + b_f                      (N,T,V,O)
  xm1 = x @ w_m1.T + b_m1 -> (N, R*T, V)     (k = r*T+t)
  xm2 = x @ w_m2.T + b_m2 -> (N, R*T, V)
  xm[k,i,j] = tanh(xm1[k,i] - xm2[k,j])
  adj[t,i,j] = alpha*(sum_k w_rm[t,k]*xm[k,i,j] + b_rm[t]) + A[t,i,j]
  out[t,i,o] = sum_j adj[t,i,j] * xf[t,j,o]

Key structural trick (avoids transposing x for the big matmuls):
  out[t] = adj[t] @ (x[t] @ w_f.T + b_f)
         = (adj[t] @ x[t]) @ w_f.T + rowsum(adj[t]) x b_f
  MM1: yT[c,i] = sum_j x[t,j,c] * adjT[j,i]   (lhsT = x[t] natural (v,c)!)
  MM2: out[i,o] = sum_c yT[c,i] * w_fT[c,o]
  With a ones-column appended to x[t], MM1 also emits rowsum(adj) as row 64
  of yT, and MM2's rhs gets b_f appended as row 64 -> bias handled exactly.

Sharding: data-parallel over batch N across 8 cores (8 n per core).
"""

import os

import numpy as np

os.environ.setdefault("BASS_NEVER_TRACE", "1")

N, T, V, C = 64, 64, 64, 64
RED, OUT = 2, 64
K = RED * T  # 128
NCORES = 8
NLOC = N // NCORES  # 8

_COMPILED = {}
_BUFS = {}


def _buf(name, shape, dtype):
    b = _BUFS.get(name)
    if b is None or b.shape != tuple(shape) or b.dtype != dtype:
        b = _BUFS[name] = np.empty(shape, dtype)
    return b


def _build():
    import concourse.bass as bass
    import concourse.tile as tile
    from concourse import bacc, bass_isa
    import concourse.mybir as mybir
    from concourse.masks import make_identity

    fp32 = mybir.dt.float32
    bf16 = mybir.dt.bfloat16
    f16 = mybir.dt.float16
    i8 = mybir.dt.int8

    nc = bacc.Bacc("TRN2", target_bir_lowering=False, debug=False, num_devices=NCORES)

    # ---- DRAM I/O ----
    xq_d = nc.dram_tensor("xq", (NLOC, T, V, C), i8, kind="ExternalInput").ap()
    xsc_d = nc.dram_tensor("xsc", (NLOC, V, T), f16, kind="ExternalInput").ap()
    a8_d = nc.dram_tensor("a8", (V, V * T), i8, kind="ExternalInput").ap()
    # packed small fp32 weights: [w_rmt (K*T) | wm_cat (C*4) | bias_tanh (K) | wfb ((C+1)*OUT)]
    wpk_d = nc.dram_tensor(
        "wpk", (1, K * T + C * 4 + K + (C + 1) * OUT), fp32, kind="ExternalInput"
    ).ap()
    # single flat output: NLOC*T*V*OUT int8 payload + 64-byte tail holding the
    # shard absmax M encoded as 5 base-100 digits (denominations 1e8..1e0 of
    # v = M*1000); the tail's unwritten bytes stay 0 via the donated zero buf
    outq_d = nc.dram_tensor(
        "outq", (1, NLOC * T * V * OUT + 64), i8, kind="ExternalOutput"
    ).ap()

    TB = C + 1  # 65: per-t block in xnat: 64 x columns + 1 ones column

    with tile.TileContext(nc) as tc:
        with (
            tc.tile_pool(name="consts", bufs=1) as consts,
            tc.tile_pool(name="outsp", bufs=1) as outsp,
            tc.tile_pool(name="work", bufs=2) as work,
            tc.tile_pool(name="work1", bufs=1) as work1,
            tc.tile_pool(name="dram", bufs=2, space="DRAM") as dram,
            tc.tile_pool(name="ps_small", bufs=2, space="PSUM") as ps_small,
            tc.tile_pool(name="ps_mv", bufs=1, space="PSUM") as ps_mv,
            tc.tile_pool(name="ps_adj", bufs=2, space="PSUM") as ps_adj,
            tc.tile_pool(name="ps_yt", bufs=2, space="PSUM") as ps_yt,
            tc.tile_pool(name="ps_out", bufs=1, space="PSUM") as ps_out,
        ):
            # ---- constants (loaded once) ----
            ident = consts.tile([64, 64], fp32, tag="ident")
            make_identity(nc, ident)
            # A_eff int8 -> fp32 via SWDGE cast DMA (values +-127; global scale
            # dA is folded into w_rmt/out-scale on the host)
            a_sb = consts.tile([V, V * T], fp32, tag="a_sb")
            nc.gpsimd.dma_start(out=a_sb, in_=a8_d)
            wpk = wpk_d[0]
            o_rm, o_wm, o_bt, o_fb = 0, K * T, K * T + C * 4, K * T + C * 4 + K
            wrm_sb = consts.tile([K, T], fp32, tag="wrm")
            nc.sync.dma_start(
                out=wrm_sb, in_=wpk[o_rm : o_rm + K * T].rearrange("(k t) -> k t", t=T)
            )
            wm_sb = consts.tile([C, 4], fp32, tag="wm")
            nc.sync.dma_start(
                out=wm_sb, in_=wpk[o_wm : o_wm + C * 4].rearrange("(c m) -> c m", m=4)
            )
            bt_sb = consts.tile([K, 1], fp32, tag="bt")
            nc.sync.dma_start(
                out=bt_sb, in_=wpk[o_bt : o_bt + K].rearrange("(k u) -> k u", u=1)
            )
            wfb_sb = consts.tile([C + 1, OUT], fp32, tag="wfb")
            nc.sync.dma_start(
                out=wfb_sb,
                in_=wpk[o_fb : o_fb + (C + 1) * OUT].rearrange("(c o) -> c o", o=OUT),
            )
            wrm_x = consts.tile([K, T], bf16, tag="wrmx")
            nc.vector.tensor_copy(wrm_x, wrm_sb)
            digs_all = consts.tile([1, 64], i8, tag="digs_all")
            nc.vector.memset(digs_all, 0)

            # warmup PE op: absorbs the gpsimd ident-wait so later matmuls
            # carry at most 2 sync waits (HW limit on LDWEIGHTS)
            warm_ps = ps_small.tile([C, 8 * V], fp32, tag="tr")
            nc.tensor.transpose(warm_ps[:, 0:C], ident, ident)

            for n in range(NLOC):
                # 1) load x[n] int8 into (v, t*65+c) layout via SWDGE cast DMA
                #    (int8 -> fp32, values +-127), then scale rows by the
                #    per-(v,t) fp32 scales; ones at col t*65+64
                xnat = work.tile([V, T * TB], fp32, tag="xnat")
                xnat_v = xnat.rearrange("v (t c) -> v t c", c=TB)
                # HWDGE int8 load into staging (RTL descriptor gen; the SWDGE
                # cast-DMA spent ~12ms/n in Q7 descriptor generation for the
                # 4096-chunk strided pattern), then DVE convert+scale
                xq_sb = work.tile([V, T * C], i8, tag="xq_sb")
                nc.sync.dma_start(
                    out=xq_sb.rearrange("v (t c) -> v t c", c=C),
                    in_=xq_d[n].rearrange("t v c -> v t c"),
                )
                xsc16 = work.tile([V, T], f16, tag="xsc16")
                nc.sync.dma_start(out=xsc16, in_=xsc_d[n])
                xsc_sb = work.tile([V, T], fp32, tag="xsc")
                nc.vector.tensor_copy(xsc_sb, xsc16)
                sc_b = bass.AP(
                    xsc_sb.tensor, xsc_sb.offset, [xsc_sb.ap[0], xsc_sb.ap[1], [0, C]]
                )
                nc.vector.tensor_tensor(
                    xnat_v[:, :, 0:C],
                    xq_sb.rearrange("v (t c) -> v t c", c=C),
                    sc_b,
                    mybir.AluOpType.mult,
                )
                nc.vector.memset(xnat_v[:, :, C : C + 1], 1.0)

                # 2) per-t transposes (8 per psum bank):
                #    xts[c, t*64+v] = x[n,t,v,c]
                xts = work1.tile([C, T * V], fp32, tag="xts")
                for q in range(T // 8):
                    tr_ps = ps_small.tile([C, 8 * V], fp32, tag="tr")
                    for tl in range(8):
                        t = q * 8 + tl
                        nc.tensor.transpose(
                            tr_ps[:, tl * V : (tl + 1) * V],
                            xnat_v[:, t, 0:C],
                            ident,
                        )
                    nc.vector.tensor_copy(xts[:, q * 512 : (q + 1) * 512], tr_ps)

                # 3) matvec: xmraw[m, t*64+v], m = [m1r0, m1r1, m2r0, m2r1]
                xmraw = work1.tile([4, T * V], fp32, tag="xmraw")
                for q in range(T * V // 512):
                    mv_ps = ps_mv.tile([4, 512], fp32, tag="mv")
                    nc.tensor.matmul(
                        mv_ps,
                        wm_sb,
                        xts[:, q * 512 : (q + 1) * 512],
                        start=True,
                        stop=True,
                    )
                    nc.vector.tensor_copy(xmraw[:, q * 512 : (q + 1) * 512], mv_ps)

                # 4) expand to xm1k/xm2k (k=(r,t) partitions, v free) via a
                #    DRAM round-trip (partition-crossing SBUF->SBUF DMAs
                #    lower to aliasing flat APs -- unsafe)
                scr = dram.tile([4, T * V], fp32, tag="scr")
                nc.sync.dma_start(out=scr, in_=xmraw)
                xm1k = work.tile([K, V], fp32, tag="xm1k")
                xm2k = work.tile([K, V], fp32, tag="xm2k")
                for dst_t, m0 in ((xm1k, 0), (xm2k, 2)):
                    nc.sync.dma_start(
                        out=dst_t,
                        in_=scr[m0 : m0 + 2].rearrange(
                            "m (t v) -> (m t) v", t=T
                        ),
                    )

                # 5+6) xm chunks (8 i at a time): negated outer-diff + tanh,
                #      then adj MMs per i; epilogue adds A_effT (int8-valued,
                #      scale folded out) into adjS
                adjs = work1.tile([V, V * T], fp32, tag="adjs")
                NCH = 8
                for ic in range(V // NCH):
                    i0 = ic * NCH
                    xmpre = work.tile([K, NCH * V], fp32, tag="xmpre")
                    in0 = bass.AP(
                        xm2k.tensor, xm2k.offset, [xm2k.ap[0], [0, NCH], xm2k.ap[1]]
                    )
                    in1 = bass.AP(
                        xm1k.tensor, xm1k.offset + i0, [xm1k.ap[0], [1, NCH], [0, V]]
                    )
                    nc.vector.tensor_tensor(
                        xmpre.rearrange("p (i j) -> p i j", i=NCH),
                        in0,
                        in1,
                        mybir.AluOpType.subtract,
                    )
                    xm_t = work.tile([K, NCH * V], bf16, tag="xm")
                    nc.scalar.activation(
                        xm_t,
                        xmpre,
                        mybir.ActivationFunctionType.Tanh,
                        bias=bt_sb,
                        scale=1.0,
                    )
                    adj_ps = ps_adj.tile([V, NCH * T], fp32, tag="adj")
                    for il in range(NCH):
                        nc.tensor.matmul(
                            adj_ps[:, il * T : (il + 1) * T],
                            xm_t[:, il * V : (il + 1) * V],
                            wrm_x,
                            start=True,
                            stop=True,
                        )
                    nc.vector.scalar_tensor_tensor(
                        adjs[:, i0 * T : (i0 + NCH) * T],
                        adj_ps,
                        1.0,
                        a_sb[:, i0 * T : (i0 + NCH) * T],
                        mybir.AluOpType.mult,
                        mybir.AluOpType.add,
                    )

                # 7) per t: MM1 -> yT (65,64) psum, copy, MM2 -> out (64,64)
                #    packed 8 t per psum bank; outs stored bf16 per n
                outs = work.tile([V, T * OUT], bf16, tag="outs")
                adjs_it = adjs.rearrange("j (i t) -> j i t", t=T)
                for tc8 in range(T // 8):
                    yt_ps = ps_yt.tile([C + 1, 8 * V], fp32, tag="yt")
                    yt_sb = work.tile([C + 1, 8 * V], fp32, tag="yt_sb")
                    for tl in range(8):
                        t = tc8 * 8 + tl
                        nc.tensor.matmul(
                            yt_ps[:, tl * V : (tl + 1) * V],
                            xnat[:, t * TB : (t + 1) * TB],
                            adjs_it[:, :, t],
                            start=True,
                            stop=True,
                        )
                    nc.vector.tensor_copy(yt_sb, yt_ps)
                    out_ps = ps_out.tile([V, 8 * OUT], fp32, tag="out")
                    for tl in range(8):
                        nc.tensor.matmul(
                            out_ps[:, tl * OUT : (tl + 1) * OUT],
                            yt_sb[:, tl * V : (tl + 1) * V],
                            wfb_sb,
                            start=True,
                            stop=True,
                        )
                    nc.scalar.copy(
                        outs[:, tc8 * 8 * OUT : (tc8 + 1) * 8 * OUT], out_ps
                    )

                # 8) per-n absmax -> scale -> digits -> quantize -> store
                #    (per-n scale keeps the output path pipelined with n+1's
                #    compute; a global scale forced a serial quantize tail)
                colmax = work.tile([V, 1], fp32, tag="colmax")
                nc.vector.tensor_reduce(
                    colmax, outs, mybir.AxisListType.X, mybir.AluOpType.max,
                    apply_absolute_value=True,
                )
                mbn = work.tile([V, 1], fp32, tag="mbn")
                nc.gpsimd.partition_all_reduce(mbn, colmax, channels=V,
                                               reduce_op=bass_isa.ReduceOp.max)
                # encode v = M_n*1000 as base-100 digits (denoms 1e8..1e0)
                # into digs_all[0, n*8 : n*8+5]
                vrem = work.tile([1, 1], fp32, tag="vrem")
                nc.vector.tensor_scalar_mul(vrem, mbn[0:1, 0:1], 1000.0)
                dtmp = work.tile([1, 1], fp32, tag="dtmp")
                dfl = work.tile([1, 1], fp32, tag="dfl")
                for j, p in enumerate([1e8, 1e6, 1e4, 1e2, 1.0]):
                    nc.vector.tensor_scalar(
                        dtmp, vrem, float(1.0 / p), -0.49999997,
                        mybir.AluOpType.mult, mybir.AluOpType.add,
                    )
                    nc.vector.tensor_copy(
                        digs_all[0:1, n * 8 + j : n * 8 + j + 1], dtmp
                    )
                    nc.vector.tensor_copy(
                        dfl, digs_all[0:1, n * 8 + j : n * 8 + j + 1]
                    )
                    nc.vector.scalar_tensor_tensor(
                        vrem, dfl, float(-p), vrem,
                        mybir.AluOpType.mult, mybir.AluOpType.add,
                    )
                rb = work.tile([V, 1], fp32, tag="rb")
                nc.vector.reciprocal(rb, mbn)
                r_sb = work.tile([V, 1], fp32, tag="r_sb")
                nc.vector.tensor_scalar_mul(r_sb, rb, 127.0)
                outq_sb = work.tile([V, T * OUT], i8, tag="outq_sb")
                nc.vector.tensor_scalar(
                    outq_sb,
                    outs,
                    r_sb[:, 0:1],
                    None,
                    mybir.AluOpType.mult,
                )
                nc.sync.dma_start(
                    out=outq_d[0, n * T * V * OUT : (n + 1) * T * V * OUT]
                    .rearrange("(t i o) -> i t o", t=T, o=OUT),
                    in_=outq_sb.rearrange("i (t o) -> i t o", t=T),
                )

            nc.sync.dma_start(
                out=outq_d[0, NLOC * T * OUT * V : NLOC * T * OUT * V + 64]
                .rearrange("(u f) -> u f", u=1),
                in_=digs_all,
            )

    nc.compile()
    return nc


def _get_compiled():
    if "nc" not in _COMPILED:
        # persistent XLA compilation cache: the execute path rebuilds its
        # jax.jit wrapper per call, so without this every call re-runs the
        # backend compile (~0.5s); with it, warm calls hit the disk cache.
        try:
            import os
            import tempfile

            import jax

            cdir = os.path.join(tempfile.gettempdir(), "jax_comp_cache")
            jax.config.update("jax_compilation_cache_dir", cdir)
            jax.config.update("jax_persistent_cache_min_compile_time_secs", 0)
            jax.config.update("jax_persistent_cache_min_entry_size_bytes", -1)
        except Exception:
            pass
        _COMPILED["nc"] = _build()
    return _COMPILED["nc"]


def _prep_inputs(A, w_m1, b_m1, w_m2, b_m2, w_rm, b_rm, w_f, b_f, alpha_m):
    f32 = np.float32
    alpha = float(alpha_m)
    # A_effT[j, i*T+t] = A[t,i,j] + alpha*b_rm[t]; int8 with global scale dA
    a_eff = np.asarray(A, f32) + (alpha * np.asarray(b_rm, f32))[:, None, None]
    a_efft = np.ascontiguousarray(a_eff.transpose(2, 1, 0).reshape(V, V * T))
    dA = max(float(np.abs(a_efft).max()), 1e-30) / 127.0
    a8 = np.rint(a_efft / dA).astype(np.int8)
    # negated+scaled w_rm (compensates the negated outer difference); the
    # 1/dA factor scales the whole adj so the device-side A add is integer-
    # valued -- undone on the host via the output scale (M * dA)
    w_rmt = np.ascontiguousarray((-alpha / dA * np.asarray(w_rm, f32)).T)  # (K, T)
    # matvec weights; cols = [m1r0, m1r1, m2r0, m2r1]
    wm_cat = np.ascontiguousarray(
        np.concatenate([np.asarray(w_m1, f32).T, np.asarray(w_m2, f32).T], axis=1)
    )  # (C, 4)
    # tanh arg = (xm2+b_m2) - (xm1+b_m1) = (xm2-xm1) + (b_m2-b_m1)
    bias_tanh = np.ascontiguousarray(
        np.repeat(np.asarray(b_m2, f32) - np.asarray(b_m1, f32), T)[:, None]
    )
    wfb = np.concatenate(
        [np.asarray(w_f, f32).T, np.asarray(b_f, f32)[None]], axis=0
    )  # (65, O)
    wpk = np.concatenate(
        [w_rmt.ravel(), wm_cat.ravel(), bias_tanh.ravel(), wfb.ravel()]
    )[None, :]
    return a8, dA, wpk


def kernel(x, A, w_m1, b_m1, w_m2, b_m2, w_rm, b_rm, w_f, b_f, alpha_m,
           _trace=False):
    from concourse import bass_utils

    a8, dA, wpk = _prep_inputs(
        A, w_m1, b_m1, w_m2, b_m2, w_rm, b_rm, w_f, b_f, alpha_m
    )
    x = np.asarray(x, np.float32)
    # per-row (n,t,v) symmetric int8 quantization of x
    tmp = _buf("tmp", x.shape, np.float32)
    np.abs(x, out=tmp)
    absrow = tmp.max(axis=-1)  # (N,T,V)
    np.maximum(absrow, np.float32(1e-30), out=absrow)
    r = np.float32(127.0) / absrow
    np.multiply(x, r[..., None], out=tmp)
    np.rint(tmp, out=tmp)
    xq = _buf("xq", x.shape, np.int8)
    np.copyto(xq, tmp, casting="unsafe")  # exact ints in [-127,127]
    d = absrow * np.float32(1.0 / 127.0)
    dsT = np.ascontiguousarray(d.transpose(0, 2, 1)).astype(np.float16)  # (N,V,T)

    in_maps = []
    for c in range(NCORES):
        in_maps.append({
            "xq": xq[c * NLOC : (c + 1) * NLOC],
            "xsc": dsT[c * NLOC : (c + 1) * NLOC],
            "a8": a8,
            "wpk": wpk,
        })
    nc = _get_compiled()
    res = bass_utils.run_bass_kernel_spmd(
        nc, in_maps, core_ids=list(range(NCORES)), trace=_trace
    )
    out = np.empty((N, T, V, OUT), np.float32)
    denom = np.array([1e8, 1e6, 1e4, 1e2, 1.0])
    for c in range(NCORES):
        flat = res.results[c]["outq"][0]
        tail = flat[NLOC * T * V * OUT :].reshape(NLOC, 8)[:, :5].astype(np.float64)
        Ms = (tail * denom).sum(axis=1) / 1000.0  # per-n absmax
        scs = (Ms * (dA / 127.0)).astype(np.float32)
        sl = out[c * NLOC : (c + 1) * NLOC]
        np.copyto(
            sl,
            flat[: NLOC * T * V * OUT].reshape(NLOC, T, V, OUT),
            casting="unsafe",
        )  # int8 -> fp32 exact
        sl *= scs[:, None, None, None]
    kernel._last_result = res
    return out



# revision 12
# speedup vs baseline: 1.0802x; 1.0177x over previous
"""Trainium2 Bass kernel for nn_DSTDGC (gnn_message_passing) — transfer-optimized.

The graded metric (wall time of kernel() on axon-tunneled cores) is dominated
by host<->device transfer (~25-40 MB/s each way, plus ~9ms per array and
~0.1s per extra output), so the kernel minimizes bytes and array count:
  - x sent as int8 with per-row (n,t,v) fp16 scales (dequantized on device)
  - A_eff sent as int8 with its global scale folded into w_rm on the host
    (the whole adj/out is uniformly scaled by 1/dA; undone via the output scale)
  - small fp32 weights packed into one flat input array
  - single int8 output: quantized out values + the device-computed shard absmax
    M encoded as base-100 digits in a 64-byte tail (avoids a second output,
    which costs ~0.1s in per-output gather latency); dequantized on the host

Math (per batch n):
  xf  = x @ w_f.T + b_f                      (N,T,V,O)
  xm1 = x @ w_m1.T + b_m1 -> (N, R*T, V)     (k = r*T+t)
  xm2 = x @ w_m2.T + b_m2 -> (N, R*T, V)
  xm[k,i,j] = tanh(xm1[k,i] - xm2[k,j])
  adj[t,i,j] = alpha*(sum_k w_rm[t,k]*xm[k,i,j] + b_rm[t]) + A[t,i,j]
  out[t,i,o] = sum_j adj[t,i,j] * xf[t,j,o]

Key structural trick (avoids transposing x for the big matmuls):
  out[t] = adj[t] @ (x[t] @ w_f.T + b_f)
         = (adj[t] @ x[t]) @ w_f.T + rowsum(adj[t]) x b_f
  MM1: yT[c,i] = sum_j x[t,j,c] * adjT[j,i]   (lhsT = x[t] natural (v,c)!)
  MM2: out[i,o] = sum_c yT[c,i] * w_fT[c,o]
  With a ones-column appended to x[t], MM1 also emits rowsum(adj) as row 64
  of yT, and MM2's rhs gets b_f appended as row 64 -> bias handled exactly.

Sharding: data-parallel over batch N across 8 cores (8 n per core).
"""

import os

import numpy as np

os.environ.setdefault("BASS_NEVER_TRACE", "1")

N, T, V, C = 64, 64, 64, 64
RED, OUT = 2, 64
K = RED * T  # 128
NCORES = 8
NLOC = N // NCORES  # 8

_COMPILED = {}
_BUFS = {}


def _buf(name, shape, dtype):
    b = _BUFS.get(name)
    if b is None or b.shape != tuple(shape) or b.dtype != dtype:
        b = _BUFS[name] = np.empty(shape, dtype)
    return b


def _build():
    import concourse.bass as bass
    import concourse.tile as tile
    from concourse import bacc, bass_isa
    import concourse.mybir as mybir
    from concourse.masks import make_identity

    fp32 = mybir.dt.float32
    bf16 = mybir.dt.bfloat16
    f16 = mybir.dt.float16
    i8 = mybir.dt.int8

    nc = bacc.Bacc("TRN2", target_bir_lowering=False, debug=False, num_devices=NCORES)

    # ---- DRAM I/O ----
    xq_d = nc.dram_tensor("xq", (NLOC, T, V, C), i8, kind="ExternalInput").ap()
    xsc_d = nc.dram_tensor("xsc", (NLOC, V, T), f16, kind="ExternalInput").ap()
    a8_d = nc.dram_tensor("a8", (V, V * T), i8, kind="ExternalInput").ap()
    # packed small fp32 weights: [w_rmt (K*T) | wm_cat (C*4) | bias_tanh (K) | wfb ((C+1)*OUT)]
    wpk_d = nc.dram_tensor(
        "wpk", (1, K * T + C * 4 + K + (C + 1) * OUT), fp32, kind="ExternalInput"
    ).ap()
    # single flat output: NLOC*T*V*OUT int8 payload + 64-byte tail holding the
    # shard absmax M encoded as 5 base-100 digits (denominations 1e8..1e0 of
    # v = M*1000); the tail's unwritten bytes stay 0 via the donated zero buf
    outq_d = nc.dram_tensor(
        "outq", (1, NLOC * T * V * OUT + 64), i8, kind="ExternalOutput"
    ).ap()

    TB = C + 1  # 65: per-t block in xnat: 64 x columns + 1 ones column

    with tile.TileContext(nc) as tc:
        with (
            tc.tile_pool(name="consts", bufs=1) as consts,
            tc.tile_pool(name="outsp", bufs=1) as outsp,
            tc.tile_pool(name="work", bufs=2) as work,
            tc.tile_pool(name="work1", bufs=2) as work1,
            tc.tile_pool(name="dram", bufs=2, space="DRAM") as dram,
            tc.tile_pool(name="ps_small", bufs=2, space="PSUM") as ps_small,
            tc.tile_pool(name="ps_mv", bufs=1, space="PSUM") as ps_mv,
            tc.tile_pool(name="ps_adj", bufs=2, space="PSUM") as ps_adj,
            tc.tile_pool(name="ps_yt", bufs=2, space="PSUM") as ps_yt,
            tc.tile_pool(name="ps_out", bufs=1, space="PSUM") as ps_out,
        ):
            # ---- constants (loaded once) ----
            ident = consts.tile([64, 64], fp32, tag="ident")
            make_identity(nc, ident)
            # A_eff int8 -> fp32 via SWDGE cast DMA (values +-127; global scale
            # dA is folded into w_rmt/out-scale on the host)
            a_sb = consts.tile([V, V * T], fp32, tag="a_sb")
            nc.gpsimd.dma_start(out=a_sb, in_=a8_d)
            wpk = wpk_d[0]
            o_rm, o_wm, o_bt, o_fb = 0, K * T, K * T + C * 4, K * T + C * 4 + K
            wrm_sb = consts.tile([K, T], fp32, tag="wrm")
            nc.sync.dma_start(
                out=wrm_sb, in_=wpk[o_rm : o_rm + K * T].rearrange("(k t) -> k t", t=T)
            )
            wm_sb = consts.tile([C, 4], fp32, tag="wm")
            nc.sync.dma_start(
                out=wm_sb, in_=wpk[o_wm : o_wm + C * 4].rearrange("(c m) -> c m", m=4)
            )
            bt_sb = consts.tile([K, 1], fp32, tag="bt")
            nc.sync.dma_start(
                out=bt_sb, in_=wpk[o_bt : o_bt + K].rearrange("(k u) -> k u", u=1)
            )
            wfb_sb = consts.tile([C + 1, OUT], fp32, tag="wfb")
            nc.sync.dma_start(
                out=wfb_sb,
                in_=wpk[o_fb : o_fb + (C + 1) * OUT].rearrange("(c o) -> c o", o=OUT),
            )
            wrm_x = consts.tile([K, T], bf16, tag="wrmx")
            nc.vector.tensor_copy(wrm_x, wrm_sb)
            digs_all = consts.tile([1, 64], i8, tag="digs_all")
            nc.vector.memset(digs_all, 0)

            # warmup PE op: absorbs the gpsimd ident-wait so later matmuls
            # carry at most 2 sync waits (HW limit on LDWEIGHTS)
            warm_ps = ps_small.tile([C, 8 * V], fp32, tag="tr")
            nc.tensor.transpose(warm_ps[:, 0:C], ident, ident)

            for n in range(NLOC):
                # 1) load x[n] int8 into (v, t*65+c) layout via SWDGE cast DMA
                #    (int8 -> fp32, values +-127), then scale rows by the
                #    per-(v,t) fp32 scales; ones at col t*65+64
                xnat = work.tile([V, T * TB], fp32, tag="xnat")
                xnat_v = xnat.rearrange("v (t c) -> v t c", c=TB)
                # HWDGE int8 load into staging (RTL descriptor gen; the SWDGE
                # cast-DMA spent ~12ms/n in Q7 descriptor generation for the
                # 4096-chunk strided pattern), then DVE convert+scale
                xq_sb = work.tile([V, T * C], i8, tag="xq_sb")
                nc.sync.dma_start(
                    out=xq_sb.rearrange("v (t c) -> v t c", c=C),
                    in_=xq_d[n].rearrange("t v c -> v t c"),
                )
                xsc16 = work.tile([V, T], f16, tag="xsc16")
                nc.sync.dma_start(out=xsc16, in_=xsc_d[n])
                xsc_sb = work.tile([V, T], fp32, tag="xsc")
                nc.vector.tensor_copy(xsc_sb, xsc16)
                sc_b = bass.AP(
                    xsc_sb.tensor, xsc_sb.offset, [xsc_sb.ap[0], xsc_sb.ap[1], [0, C]]
                )
                nc.vector.tensor_tensor(
                    xnat_v[:, :, 0:C],
                    xq_sb.rearrange("v (t c) -> v t c", c=C),
                    sc_b,
                    mybir.AluOpType.mult,
                )
                nc.vector.memset(xnat_v[:, :, C : C + 1], 1.0)

                # 2) per-t transposes (8 per psum bank):
                #    xts[c, t*64+v] = x[n,t,v,c]
                xts = work1.tile([C, T * V], fp32, tag="xts")
                for q in range(T // 8):
                    tr_ps = ps_small.tile([C, 8 * V], fp32, tag="tr")
                    for tl in range(8):
                        t = q * 8 + tl
                        nc.tensor.transpose(
                            tr_ps[:, tl * V : (tl + 1) * V],
                            xnat_v[:, t, 0:C],
                            ident,
                        )
                    nc.vector.tensor_copy(xts[:, q * 512 : (q + 1) * 512], tr_ps)

                # 3) matvec: xmraw[m, t*64+v], m = [m1r0, m1r1, m2r0, m2r1]
                xmraw = work1.tile([4, T * V], fp32, tag="xmraw")
                for q in range(T * V // 512):
                    mv_ps = ps_mv.tile([4, 512], fp32, tag="mv")
                    nc.tensor.matmul(
                        mv_ps,
                        wm_sb,
                        xts[:, q * 512 : (q + 1) * 512],
                        start=True,
                        stop=True,
                    )
                    nc.vector.tensor_copy(xmraw[:, q * 512 : (q + 1) * 512], mv_ps)

                # 4) expand to xm1k/xm2k (k=(r,t) partitions, v free) via a
                #    DRAM round-trip (partition-crossing SBUF->SBUF DMAs
                #    lower to aliasing flat APs -- unsafe)
                scr = dram.tile([4, T * V], fp32, tag="scr")
                nc.sync.dma_start(out=scr, in_=xmraw)
                xm1k = work.tile([K, V], fp32, tag="xm1k")
                xm2k = work.tile([K, V], fp32, tag="xm2k")
                for dst_t, m0 in ((xm1k, 0), (xm2k, 2)):
                    nc.sync.dma_start(
                        out=dst_t,
                        in_=scr[m0 : m0 + 2].rearrange(
                            "m (t v) -> (m t) v", t=T
                        ),
                    )

                # 5+6) xm chunks (8 i at a time): negated outer-diff + tanh,
                #      then adj MMs per i; epilogue adds A_effT (int8-valued,
                #      scale folded out) into adjS
                adjs = work1.tile([V, V * T], fp32, tag="adjs")
                NCH = 8
                for ic in range(V // NCH):
                    i0 = ic * NCH
                    xmpre = work.tile([K, NCH * V], fp32, tag="xmpre")
                    in0 = bass.AP(
                        xm2k.tensor, xm2k.offset, [xm2k.ap[0], [0, NCH], xm2k.ap[1]]
                    )
                    in1 = bass.AP(
                        xm1k.tensor, xm1k.offset + i0, [xm1k.ap[0], [1, NCH], [0, V]]
                    )
                    nc.vector.tensor_tensor(
                        xmpre.rearrange("p (i j) -> p i j", i=NCH),
                        in0,
                        in1,
                        mybir.AluOpType.subtract,
                    )
                    xm_t = work.tile([K, NCH * V], bf16, tag="xm")
                    nc.scalar.activation(
                        xm_t,
                        xmpre,
                        mybir.ActivationFunctionType.Tanh,
                        bias=bt_sb,
                        scale=1.0,
                    )
                    adj_ps = ps_adj.tile([V, NCH * T], fp32, tag="adj")
                    for il in range(NCH):
                        nc.tensor.matmul(
                            adj_ps[:, il * T : (il + 1) * T],
                            xm_t[:, il * V : (il + 1) * V],
                            wrm_x,
                            start=True,
                            stop=True,
                        )
                    nc.vector.scalar_tensor_tensor(
                        adjs[:, i0 * T : (i0 + NCH) * T],
                        adj_ps,
                        1.0,
                        a_sb[:, i0 * T : (i0 + NCH) * T],
                        mybir.AluOpType.mult,
                        mybir.AluOpType.add,
                    )

                # 7) per t: MM1 -> yT (65,64) psum, copy, MM2 -> out (64,64)
                #    packed 8 t per psum bank; outs stored bf16 per n
                outs = work.tile([V, T * OUT], bf16, tag="outs")
                adjs_it = adjs.rearrange("j (i t) -> j i t", t=T)
                for tc8 in range(T // 8):
                    yt_ps = ps_yt.tile([C + 1, 8 * V], fp32, tag="yt")
                    yt_sb = work.tile([C + 1, 8 * V], fp32, tag="yt_sb")
                    for tl in range(8):
                        t = tc8 * 8 + tl
                        nc.tensor.matmul(
                            yt_ps[:, tl * V : (tl + 1) * V],
                            xnat[:, t * TB : (t + 1) * TB],
                            adjs_it[:, :, t],
                            start=True,
                            stop=True,
                        )
                    nc.vector.tensor_copy(yt_sb, yt_ps)
                    out_ps = ps_out.tile([V, 8 * OUT], fp32, tag="out")
                    for tl in range(8):
                        nc.tensor.matmul(
                            out_ps[:, tl * OUT : (tl + 1) * OUT],
                            yt_sb[:, tl * V : (tl + 1) * V],
                            wfb_sb,
                            start=True,
                            stop=True,
                        )
                    nc.scalar.copy(
                        outs[:, tc8 * 8 * OUT : (tc8 + 1) * 8 * OUT], out_ps
                    )

                # 8) per-n absmax -> scale -> digits -> quantize -> store
                #    (per-n scale keeps the output path pipelined with n+1's
                #    compute; a global scale forced a serial quantize tail)
                colmax = work.tile([V, 1], fp32, tag="colmax")
                nc.vector.tensor_reduce(
                    colmax, outs, mybir.AxisListType.X, mybir.AluOpType.max,
                    apply_absolute_value=True,
                )
                mbn = work.tile([V, 1], fp32, tag="mbn")
                nc.gpsimd.partition_all_reduce(mbn, colmax, channels=V,
                                               reduce_op=bass_isa.ReduceOp.max)
                # encode v = M_n*1000 as base-100 digits (denoms 1e8..1e0)
                # into digs_all[0, n*8 : n*8+5]
                vrem = work.tile([1, 1], fp32, tag="vrem")
                nc.vector.tensor_scalar_mul(vrem, mbn[0:1, 0:1], 1000.0)
                dtmp = work.tile([1, 1], fp32, tag="dtmp")
                dfl = work.tile([1, 1], fp32, tag="dfl")
                for j, p in enumerate([1e8, 1e6, 1e4, 1e2, 1.0]):
                    nc.vector.tensor_scalar(
                        dtmp, vrem, float(1.0 / p), -0.49999997,
                        mybir.AluOpType.mult, mybir.AluOpType.add,
                    )
                    nc.vector.tensor_copy(
                        digs_all[0:1, n * 8 + j : n * 8 + j + 1], dtmp
                    )
                    nc.vector.tensor_copy(
                        dfl, digs_all[0:1, n * 8 + j : n * 8 + j + 1]
                    )
                    nc.vector.scalar_tensor_tensor(
                        vrem, dfl, float(-p), vrem,
                        mybir.AluOpType.mult, mybir.AluOpType.add,
                    )
                rb = work.tile([V, 1], fp32, tag="rb")
                nc.vector.reciprocal(rb, mbn)
                r_sb = work.tile([V, 1], fp32, tag="r_sb")
                nc.vector.tensor_scalar_mul(r_sb, rb, 127.0)
                outq_sb = work.tile([V, T * OUT], i8, tag="outq_sb")
                nc.vector.tensor_scalar(
                    outq_sb,
                    outs,
                    r_sb[:, 0:1],
                    None,
                    mybir.AluOpType.mult,
                )
                nc.sync.dma_start(
                    out=outq_d[0, n * T * V * OUT : (n + 1) * T * V * OUT]
                    .rearrange("(t i o) -> i t o", t=T, o=OUT),
                    in_=outq_sb.rearrange("i (t o) -> i t o", t=T),
                )

            nc.sync.dma_start(
                out=outq_d[0, NLOC * T * OUT * V : NLOC * T * OUT * V + 64]
                .rearrange("(u f) -> u f", u=1),
                in_=digs_all,
            )

    nc.compile()
    return nc


def _get_compiled():
    if "nc" not in _COMPILED:
        # persistent XLA compilation cache: the execute path rebuilds its
        # jax.jit wrapper per call, so without this every call re-runs the
        # backend compile (~0.5s); with it, warm calls hit the disk cache.
        try:
            import os
            import tempfile

            import jax

            cdir = os.path.join(tempfile.gettempdir(), "jax_comp_cache")
            jax.config.update("jax_compilation_cache_dir", cdir)
            jax.config.update("jax_persistent_cache_min_compile_time_secs", 0)
            jax.config.update("jax_persistent_cache_min_entry_size_bytes", -1)
        except Exception:
            pass
        _COMPILED["nc"] = _build()
    return _COMPILED["nc"]


def _prep_inputs(A, w_m1, b_m1, w_m2, b_m2, w_rm, b_rm, w_f, b_f, alpha_m):
    f32 = np.float32
    alpha = float(alpha_m)
    # A_effT[j, i*T+t] = A[t,i,j] + alpha*b_rm[t]; int8 with global scale dA
    a_eff = np.asarray(A, f32) + (alpha * np.asarray(b_rm, f32))[:, None, None]
    a_efft = np.ascontiguousarray(a_eff.transpose(2, 1, 0).reshape(V, V * T))
    dA = max(float(np.abs(a_efft).max()), 1e-30) / 127.0
    a8 = np.rint(a_efft / dA).astype(np.int8)
    # negated+scaled w_rm (compensates the negated outer difference); the
    # 1/dA factor scales the whole adj so the device-side A add is integer-
    # valued -- undone on the host via the output scale (M * dA)
    w_rmt = np.ascontiguousarray((-alpha / dA * np.asarray(w_rm, f32)).T)  # (K, T)
    # matvec weights; cols = [m1r0, m1r1, m2r0, m2r1]
    wm_cat = np.ascontiguousarray(
        np.concatenate([np.asarray(w_m1, f32).T, np.asarray(w_m2, f32).T], axis=1)
    )  # (C, 4)
    # tanh arg = (xm2+b_m2) - (xm1+b_m1) = (xm2-xm1) + (b_m2-b_m1)
    bias_tanh = np.ascontiguousarray(
        np.repeat(np.asarray(b_m2, f32) - np.asarray(b_m1, f32), T)[:, None]
    )
    wfb = np.concatenate(
        [np.asarray(w_f, f32).T, np.asarray(b_f, f32)[None]], axis=0
    )  # (65, O)
    wpk = np.concatenate(
        [w_rmt.ravel(), wm_cat.ravel(), bias_tanh.ravel(), wfb.ravel()]
    )[None, :]
    return a8, dA, wpk


def kernel(x, A, w_m1, b_m1, w_m2, b_m2, w_rm, b_rm, w_f, b_f, alpha_m,
           _trace=False):
    from concourse import bass_utils

    a8, dA, wpk = _prep_inputs(
        A, w_m1, b_m1, w_m2, b_m2, w_rm, b_rm, w_f, b_f, alpha_m
    )
    x = np.asarray(x, np.float32)
    # per-row (n,t,v) symmetric int8 quantization of x
    tmp = _buf("tmp", x.shape, np.float32)
    np.abs(x, out=tmp)
    absrow = tmp.max(axis=-1)  # (N,T,V)
    np.maximum(absrow, np.float32(1e-30), out=absrow)
    r = np.float32(127.0) / absrow
    np.multiply(x, r[..., None], out=tmp)
    np.rint(tmp, out=tmp)
    xq = _buf("xq", x.shape, np.int8)
    np.copyto(xq, tmp, casting="unsafe")  # exact ints in [-127,127]
    d = absrow * np.float32(1.0 / 127.0)
    dsT = np.ascontiguousarray(d.transpose(0, 2, 1)).astype(np.float16)  # (N,V,T)

    in_maps = []
    for c in range(NCORES):
        in_maps.append({
            "xq": xq[c * NLOC : (c + 1) * NLOC],
            "xsc": dsT[c * NLOC : (c + 1) * NLOC],
            "a8": a8,
            "wpk": wpk,
        })
    nc = _get_compiled()
    res = bass_utils.run_bass_kernel_spmd(
        nc, in_maps, core_ids=list(range(NCORES)), trace=_trace
    )
    out = np.empty((N, T, V, OUT), np.float32)
    denom = np.array([1e8, 1e6, 1e4, 1e2, 1.0])
    for c in range(NCORES):
        flat = res.results[c]["outq"][0]
        tail = flat[NLOC * T * V * OUT :].reshape(NLOC, 8)[:, :5].astype(np.float64)
        Ms = (tail * denom).sum(axis=1) / 1000.0  # per-n absmax
        scs = (Ms * (dA / 127.0)).astype(np.float32)
        sl = out[c * NLOC : (c + 1) * NLOC]
        np.copyto(
            sl,
            flat[: NLOC * T * V * OUT].reshape(NLOC, T, V, OUT),
            casting="unsafe",
        )  # int8 -> fp32 exact
        sl *= scs[:, None, None, None]
    kernel._last_result = res
    return out

